# revision 26
# baseline (speedup 1.0000x reference)
"""Trainium2 Bass kernel for nn_CoAttentionFusionBlock.

Math: the reference's softmax is over a singleton dim, so its weights are
exactly 1.0 and o1/o2 equal the raw features bit-for-bit. The module reduces to

    out = concat([feat_depth, feat_rgb], axis=1) @ W_f.T + b_f        # [B, D]

W_k1/b_k1/W_k2/b_k2 only feed the (dead) score path and are never needed.

Distribution: pure data parallel over the batch dim across 8 NeuronCores.
Each core computes yT = WfT.T @ xT (all operands pre-transposed on host so
the contraction dim lands on SBUF partitions), where
    xT  = concat([feat_depth, feat_rgb], 1).T shard   [2048, 4096]
    WfT = W_f.T                                       [2048, 1024]
    yT  = out shard transposed                        [1024, 4096]

Matmul inputs are declared float32r (fp32 bits, single-pass TF32-style PE
matmul at 1 cycle/row for moving dim >= 256) so the PE runs at full rate
instead of the 4x-slower exact-fp32 hi/lo decomposition. Measured ~260us/core
HW, absmax rel err ~1.6e-4 vs the fp32 reference.
"""

import numpy as np

import concourse.bacc as bacc
import concourse.mybir as mybir
import concourse.tile as tile
from concourse.bass_utils import run_bass_kernel_spmd

B = 32768
D = 1024
NCORES = 8
BLOC = B // NCORES  # 4096 rows per core
K = 2 * D  # 2048 contraction dim
P = 128  # partitions
NT = 512  # moving free dim per matmul (one PSUM bank of fp32)
KT = K // P  # 16 k-tiles
JT = D // P  # 8 output-row tiles
BT = BLOC // NT  # 8 batch tiles

FP32 = mybir.dt.float32
FP32R = mybir.dt.float32r

# test.py can flip these to profile; harness leaves them alone.
TRACE = False
TRACE_DIR = None
LAST_RESULT = None

# Matmul input dtype: "fp32r" (exact fp32 bits, TF32-ish multiply) or "bf16"
# (half the DMA traffic, 2 cols/cycle on the PE) or "mixed" (fp32r weights,
# bf16 activations).
DT_IN = "fp32r"


def _dtypes():
    if DT_IN == "fp32r":
        return FP32R, FP32R, np.float32, np.float32
    import ml_dtypes

    bf16 = np.dtype(ml_dtypes.bfloat16)
    if DT_IN == "bf16":
        return mybir.dt.bfloat16, mybir.dt.bfloat16, bf16, bf16
    if DT_IN == "mixed":
        return FP32R, mybir.dt.bfloat16, np.float32, bf16
    raise ValueError(DT_IN)


def _build_nc():
    # Bacc (not raw Bass): its compile() runs move_matmul_waits_to_ldweights +
    # generate_event_semaphores, which split sync waits to <=1 per instruction
    # (TRN2 HW limit — raw Bass hits "Too many sync wait commands" in walrus).
    nc = bacc.Bacc(None)
    w_dt, x_dt, _, _ = _dtypes()
    xT = nc.declare_dram_parameter("xT", [K, BLOC], x_dt, isOutput=False)
    wT = nc.declare_dram_parameter("wT", [K, D], w_dt, isOutput=False)
    biasT = nc.declare_dram_parameter("biasT", [P, JT], FP32, isOutput=False)
    yT = nc.declare_dram_parameter("yT", [D, BLOC], FP32, isOutput=True)

    # DRAM views with the 128-partition tile dim explicit
    xT_v = xT.rearrange("(t p) b -> p t b", p=P)  # [128, KT, BLOC]
    wT_v = wT.rearrange("(t p) j -> p t j", p=P)  # [128, KT, D]
    yT_v = yT.rearrange("(j p) b -> j p b", p=P)  # [JT, 128, BLOC]

    with tile.TileContext(nc) as tc:
        with (
            tc.tile_pool(name="wpool", bufs=1) as wpool,
            tc.tile_pool(name="xpool", bufs=2) as xpool,
            tc.tile_pool(name="opool", bufs=4) as opool,
            tc.tile_pool(name="bpool", bufs=1) as bpool,
            tc.tile_pool(name="psum", bufs=8, space="PSUM") as psum_pool,
        ):
            # Whole weight matrix resident in SBUF: [128, KT*D] fp32 = 64KB/partition.
            # One DMA per k-tile so each matmul waits on at most one DMA queue.
            # Interleave slab-0 x DMAs with the weight DMAs so the PE can start
            # ~2us in instead of waiting for all 8.4MB of weights first.
            w_sb = wpool.tile([P, KT * D], w_dt)
            x_sb0 = xpool.tile([P, KT * NT], x_dt)
            for t in range(KT):
                nc.sync.dma_start(
                    out=x_sb0[:, t * NT : (t + 1) * NT], in_=xT_v[:, t, 0:NT]
                )
                nc.sync.dma_start(out=w_sb[:, t * D : (t + 1) * D], in_=wT_v[:, t, :])
            bias_sb = bpool.tile([P, JT], FP32)
            nc.sync.dma_start(out=bias_sb[:], in_=biasT[:, :])

            def store(j, bi, ps):
                o_sb = opool.tile([P, NT], FP32)
                nc.vector.tensor_scalar_add(o_sb[:], ps[:], bias_sb[:, j : j + 1])
                nc.sync.dma_start(out=yT_v[j, :, bi * NT : (bi + 1) * NT], in_=o_sb[:])

            # Slab 0: k-outer with all 8 psum banks open, so each arriving
            # (w, x) k-slice pair immediately feeds 8 matmuls.
            ps0 = [psum_pool.tile([P, NT], FP32, tag="ps", name="ps") for _ in range(JT)]
            for t in range(KT):
                for j in range(JT):
                    nc.tensor.matmul(
                        ps0[j][:],
                        w_sb[:, t * D + j * P : t * D + (j + 1) * P],
                        x_sb0[:, t * NT : (t + 1) * NT],
                        start=(t == 0),
                        stop=(t == KT - 1),
                    )
            for j in range(JT):
                store(j, 0, ps0[j])

            # Remaining slabs: j-outer, psum banks recycle smoothly while the
            # next slab streams in.
            for bi in range(1, BT):
                x_sb = xpool.tile([P, KT * NT], x_dt, tag="x_sb0")
                for t in range(KT):
                    nc.sync.dma_start(
                        out=x_sb[:, t * NT : (t + 1) * NT],
                        in_=xT_v[:, t, bi * NT : (bi + 1) * NT],
                    )
                for j in range(JT):
                    ps = psum_pool.tile([P, NT], FP32, tag="ps")
                    for t in range(KT):
                        nc.tensor.matmul(
                            ps[:],
                            w_sb[:, t * D + j * P : t * D + (j + 1) * P],
                            x_sb[:, t * NT : (t + 1) * NT],
                            start=(t == 0),
                            stop=(t == KT - 1),
                        )
                    store(j, bi, ps)
    nc.finalize()
    return nc


def kernel(feat_rgb, feat_depth, W_k1, b_k1, W_k2, b_k2, W_f, b_f):
    global LAST_RESULT
    feat_rgb = np.asarray(feat_rgb, dtype=np.float32)
    feat_depth = np.asarray(feat_depth, dtype=np.float32)
    W_f = np.asarray(W_f, dtype=np.float32)
    b_f = np.asarray(b_f, dtype=np.float32)

    _, _, w_np, x_np = _dtypes()
    WfT = np.ascontiguousarray(W_f.T).astype(w_np)  # [2048, 1024]
    biasT = np.ascontiguousarray(b_f.reshape(JT, P).T)  # [128, 8]

    in_maps = []
    for i in range(NCORES):
        lo, hi = i * BLOC, (i + 1) * BLOC
        x_cat_T = np.empty((K, BLOC), dtype=x_np)
        x_cat_T[:D] = feat_depth[lo:hi].T
        x_cat_T[D:] = feat_rgb[lo:hi].T
        in_maps.append({"xT": x_cat_T, "wT": WfT, "biasT": biasT})

    nc = _build_nc()
    res = run_bass_kernel_spmd(
        nc, in_maps, list(range(NCORES)), trace=TRACE, tmpdir=TRACE_DIR
    )
    LAST_RESULT = res

    out = np.empty((B, D), dtype=np.float32)
    for i in range(NCORES):
        out[i * BLOC : (i + 1) * BLOC] = res.results[i]["yT"].T
    return out



# revision 27
# speedup vs baseline: 1.0278x; 1.0278x over previous
"""Trainium2 Bass kernel for nn_CoAttentionFusionBlock.

Math: the reference's softmax is over a singleton dim, so its weights are
exactly 1.0 and o1/o2 equal the raw features bit-for-bit. The module reduces to

    out = concat([feat_depth, feat_rgb], axis=1) @ W_f.T + b_f        # [B, D]

W_k1/b_k1/W_k2/b_k2 only feed the (dead) score path and are never needed.

Distribution: pure data parallel over the batch dim across 8 NeuronCores.
Each core computes yT = WfT.T @ xT (all operands pre-transposed on host so
the contraction dim lands on SBUF partitions), where
    xT  = concat([feat_depth, feat_rgb], 1).T shard   [2048, 4096]
    WfT = W_f.T                                       [2048, 1024]
    yT  = out shard transposed                        [1024, 4096]

Matmul inputs are declared float32r (fp32 bits, single-pass TF32-style PE
matmul at 1 cycle/row for moving dim >= 256) so the PE runs at full rate
instead of the 4x-slower exact-fp32 hi/lo decomposition. Measured ~260us/core
HW, absmax rel err ~1.6e-4 vs the fp32 reference.
"""

import numpy as np

import concourse.bacc as bacc
import concourse.mybir as mybir
import concourse.tile as tile
from concourse.bass_utils import run_bass_kernel_spmd

B = 32768
D = 1024
NCORES = 8
BLOC = B // NCORES  # 4096 rows per core
K = 2 * D  # 2048 contraction dim
P = 128  # partitions
NT = 512  # moving free dim per matmul (one PSUM bank of fp32)
KT = K // P  # 16 k-tiles
JT = D // P  # 8 output-row tiles
BT = BLOC // NT  # 8 batch tiles

FP32 = mybir.dt.float32
FP32R = mybir.dt.float32r

# test.py can flip these to profile; harness leaves them alone.
TRACE = False
TRACE_DIR = None
LAST_RESULT = None

# Matmul input dtype: "fp32r" (exact fp32 bits, TF32-ish multiply) or "bf16"
# (half the DMA traffic, 2 cols/cycle on the PE) or "mixed" (fp32r weights,
# bf16 activations).
DT_IN = "fp32r"


def _dtypes():
    if DT_IN == "fp32r":
        return FP32R, FP32R, np.float32, np.float32
    import ml_dtypes

    bf16 = np.dtype(ml_dtypes.bfloat16)
    if DT_IN == "bf16":
        return mybir.dt.bfloat16, mybir.dt.bfloat16, bf16, bf16
    if DT_IN == "mixed":
        return FP32R, mybir.dt.bfloat16, np.float32, bf16
    raise ValueError(DT_IN)


def _build_nc():
    # Bacc (not raw Bass): its compile() runs move_matmul_waits_to_ldweights +
    # generate_event_semaphores, which split sync waits to <=1 per instruction
    # (TRN2 HW limit — raw Bass hits "Too many sync wait commands" in walrus).
    nc = bacc.Bacc(None)
    w_dt, x_dt, _, _ = _dtypes()
    xT = nc.declare_dram_parameter("xT", [K, BLOC], x_dt, isOutput=False)
    wT = nc.declare_dram_parameter("wT", [K, D], w_dt, isOutput=False)
    biasT = nc.declare_dram_parameter("biasT", [P, JT], FP32, isOutput=False)
    yT = nc.declare_dram_parameter("yT", [D, BLOC], FP32, isOutput=True)

    # DRAM views with the 128-partition tile dim explicit
    xT_v = xT.rearrange("(t p) b -> p t b", p=P)  # [128, KT, BLOC]
    wT_v = wT.rearrange("(t p) j -> p t j", p=P)  # [128, KT, D]
    yT_v = yT.rearrange("(j p) b -> j p b", p=P)  # [JT, 128, BLOC]

    with tile.TileContext(nc) as tc:
        with (
            tc.tile_pool(name="wpool", bufs=1) as wpool,
            tc.tile_pool(name="xpool", bufs=2) as xpool,
            tc.tile_pool(name="opool", bufs=4) as opool,
            tc.tile_pool(name="bpool", bufs=1) as bpool,
            tc.tile_pool(name="psum", bufs=8, space="PSUM") as psum_pool,
        ):
            # Whole weight matrix resident in SBUF: [128, KT*D] fp32 = 64KB/partition.
            # One DMA per k-tile so each matmul waits on at most one DMA queue.
            # Interleave slab-0 x DMAs with the weight DMAs so the PE can start
            # ~2us in instead of waiting for all 8.4MB of weights first.
            w_sb = wpool.tile([P, KT * D], w_dt)
            x_sb0 = xpool.tile([P, KT * NT], x_dt)
            # First k-tile fine-grained (x halves, w by j) so the first real
            # matmul's operands land in ~2us and the HAM clock-gate warms
            # while the bulk still streams.
            nc.sync.dma_start(out=x_sb0[:, 0 : NT // 2], in_=xT_v[:, 0, 0 : NT // 2])
            nc.sync.dma_start(out=w_sb[:, 0:P], in_=wT_v[:, 0, 0:P])
            nc.sync.dma_start(out=x_sb0[:, NT // 2 : NT], in_=xT_v[:, 0, NT // 2 : NT])
            for j in range(1, JT):
                nc.sync.dma_start(
                    out=w_sb[:, j * P : (j + 1) * P],
                    in_=wT_v[:, 0, j * P : (j + 1) * P],
                )
            for t in range(1, KT):
                nc.sync.dma_start(
                    out=x_sb0[:, t * NT : (t + 1) * NT], in_=xT_v[:, t, 0:NT]
                )
                nc.sync.dma_start(out=w_sb[:, t * D : (t + 1) * D], in_=wT_v[:, t, :])
            bias_sb = bpool.tile([P, JT], FP32)
            nc.sync.dma_start(out=bias_sb[:], in_=biasT[:, :])

            def store(j, bi, ps):
                o_sb = opool.tile([P, NT], FP32)
                nc.vector.tensor_scalar_add(o_sb[:], ps[:], bias_sb[:, j : j + 1])
                nc.sync.dma_start(out=yT_v[j, :, bi * NT : (bi + 1) * NT], in_=o_sb[:])

            # Slab 0: k-outer with all 8 psum banks open, so each arriving
            # (w, x) k-slice pair immediately feeds 8 matmuls.
            ps0 = [psum_pool.tile([P, NT], FP32, tag="ps", name="ps") for _ in range(JT)]
            for t in range(KT):
                for j in range(JT):
                    nc.tensor.matmul(
                        ps0[j][:],
                        w_sb[:, t * D + j * P : t * D + (j + 1) * P],
                        x_sb0[:, t * NT : (t + 1) * NT],
                        start=(t == 0),
                        stop=(t == KT - 1),
                    )
            for j in range(JT):
                store(j, 0, ps0[j])

            # Remaining slabs: j-outer, psum banks recycle smoothly while the
            # next slab streams in.
            for bi in range(1, BT):
                x_sb = xpool.tile([P, KT * NT], x_dt, tag="x_sb0")
                for t in range(KT):
                    nc.sync.dma_start(
                        out=x_sb[:, t * NT : (t + 1) * NT],
                        in_=xT_v[:, t, bi * NT : (bi + 1) * NT],
                    )
                for j in range(JT):
                    ps = psum_pool.tile([P, NT], FP32, tag="ps")
                    for t in range(KT):
                        nc.tensor.matmul(
                            ps[:],
                            w_sb[:, t * D + j * P : t * D + (j + 1) * P],
                            x_sb[:, t * NT : (t + 1) * NT],
                            start=(t == 0),
                            stop=(t == KT - 1),
                        )
                    store(j, bi, ps)
    nc.finalize()
    return nc


def kernel(feat_rgb, feat_depth, W_k1, b_k1, W_k2, b_k2, W_f, b_f):
    global LAST_RESULT
    feat_rgb = np.asarray(feat_rgb, dtype=np.float32)
    feat_depth = np.asarray(feat_depth, dtype=np.float32)
    W_f = np.asarray(W_f, dtype=np.float32)
    b_f = np.asarray(b_f, dtype=np.float32)

    _, _, w_np, x_np = _dtypes()
    WfT = np.ascontiguousarray(W_f.T).astype(w_np)  # [2048, 1024]
    biasT = np.ascontiguousarray(b_f.reshape(JT, P).T)  # [128, 8]

    in_maps = []
    for i in range(NCORES):
        lo, hi = i * BLOC, (i + 1) * BLOC
        x_cat_T = np.empty((K, BLOC), dtype=x_np)
        x_cat_T[:D] = feat_depth[lo:hi].T
        x_cat_T[D:] = feat_rgb[lo:hi].T
        in_maps.append({"xT": x_cat_T, "wT": WfT, "biasT": biasT})

    nc = _build_nc()
    res = run_bass_kernel_spmd(
        nc, in_maps, list(range(NCORES)), trace=TRACE, tmpdir=TRACE_DIR
    )
    LAST_RESULT = res

    out = np.empty((B, D), dtype=np.float32)
    for i in range(NCORES):
        out[i * BLOC : (i + 1) * BLOC] = res.results[i]["yT"].T
    return out



# revision 29
# speedup vs baseline: 1.0401x; 1.0119x over previous
"""Trainium2 Bass kernel for nn_CoAttentionFusionBlock.

Math: the reference's softmax is over a singleton dim, so its weights are
exactly 1.0 and o1/o2 equal the raw features bit-for-bit. The module reduces to

    out = concat([feat_depth, feat_rgb], axis=1) @ W_f.T + b_f        # [B, D]

W_k1/b_k1/W_k2/b_k2 only feed the (dead) score path and are never needed.

Distribution: pure data parallel over the batch dim across 8 NeuronCores.
Each core computes yT = WfT.T @ xT (all operands pre-transposed on host so
the contraction dim lands on SBUF partitions), where
    xT  = concat([feat_depth, feat_rgb], 1).T shard   [2048, 4096]
    WfT = W_f.T                                       [2048, 1024]
    yT  = out shard transposed                        [1024, 4096]

Matmul inputs are declared float32r (fp32 bits, single-pass TF32-style PE
matmul at 1 cycle/row for moving dim >= 256) so the PE runs at full rate
instead of the 4x-slower exact-fp32 hi/lo decomposition. Measured ~260us/core
HW, absmax rel err ~1.6e-4 vs the fp32 reference.
"""

import numpy as np

import concourse.bacc as bacc
import concourse.mybir as mybir
import concourse.tile as tile
from concourse.bass_utils import run_bass_kernel_spmd

B = 32768
D = 1024
NCORES = 8
BLOC = B // NCORES  # 4096 rows per core
K = 2 * D  # 2048 contraction dim
P = 128  # partitions
NT = 512  # moving free dim per matmul (one PSUM bank of fp32)
KT = K // P  # 16 k-tiles
JT = D // P  # 8 output-row tiles
BT = BLOC // NT  # 8 batch tiles

FP32 = mybir.dt.float32
FP32R = mybir.dt.float32r

# test.py can flip these to profile; harness leaves them alone.
TRACE = False
TRACE_DIR = None
LAST_RESULT = None

# Matmul input dtype: "fp32r" (exact fp32 bits, TF32-ish multiply) or "bf16"
# (half the DMA traffic, 2 cols/cycle on the PE) or "mixed" (fp32r weights,
# bf16 activations).
DT_IN = "fp32r"


def _dtypes():
    if DT_IN == "fp32r":
        return FP32R, FP32R, np.float32, np.float32
    import ml_dtypes

    bf16 = np.dtype(ml_dtypes.bfloat16)
    if DT_IN == "bf16":
        return mybir.dt.bfloat16, mybir.dt.bfloat16, bf16, bf16
    if DT_IN == "mixed":
        return FP32R, mybir.dt.bfloat16, np.float32, bf16
    raise ValueError(DT_IN)


def _build_nc():
    # Bacc (not raw Bass): its compile() runs move_matmul_waits_to_ldweights +
    # generate_event_semaphores, which split sync waits to <=1 per instruction
    # (TRN2 HW limit — raw Bass hits "Too many sync wait commands" in walrus).
    nc = bacc.Bacc(None)
    w_dt, x_dt, _, _ = _dtypes()
    xT = nc.declare_dram_parameter("xT", [K, BLOC], x_dt, isOutput=False)
    wT = nc.declare_dram_parameter("wT", [K, D], w_dt, isOutput=False)
    biasT = nc.declare_dram_parameter("biasT", [P, JT], FP32, isOutput=False)
    yT = nc.declare_dram_parameter("yT", [D, BLOC], FP32, isOutput=True)

    # DRAM views with the 128-partition tile dim explicit
    xT_v = xT.rearrange("(t p) b -> p t b", p=P)  # [128, KT, BLOC]
    wT_v = wT.rearrange("(t p) j -> p t j", p=P)  # [128, KT, D]
    yT_v = yT.rearrange("(j p) b -> j p b", p=P)  # [JT, 128, BLOC]

    with tile.TileContext(nc) as tc:
        with (
            tc.tile_pool(name="wpool", bufs=1) as wpool,
            tc.tile_pool(name="xpool", bufs=3) as xpool,
            tc.tile_pool(name="opool", bufs=4) as opool,
            tc.tile_pool(name="bpool", bufs=1) as bpool,
            tc.tile_pool(name="psum", bufs=8, space="PSUM") as psum_pool,
        ):
            # Whole weight matrix resident in SBUF: [128, KT*D] fp32 = 64KB/partition.
            # One DMA per k-tile so each matmul waits on at most one DMA queue.
            # Interleave slab-0 x DMAs with the weight DMAs so the PE can start
            # ~2us in instead of waiting for all 8.4MB of weights first.
            w_sb = wpool.tile([P, KT * D], w_dt)
            x_sb0 = xpool.tile([P, KT * NT], x_dt)
            for t in range(KT):
                nc.sync.dma_start(
                    out=x_sb0[:, t * NT : (t + 1) * NT], in_=xT_v[:, t, 0:NT]
                )
                nc.sync.dma_start(out=w_sb[:, t * D : (t + 1) * D], in_=wT_v[:, t, :])
            bias_sb = bpool.tile([P, JT], FP32)
            nc.sync.dma_start(out=bias_sb[:], in_=biasT[:, :])

            def store(j, bi, ps):
                o_sb = opool.tile([P, NT], FP32)
                nc.vector.tensor_scalar_add(o_sb[:], ps[:], bias_sb[:, j : j + 1])
                nc.sync.dma_start(out=yT_v[j, :, bi * NT : (bi + 1) * NT], in_=o_sb[:])

            # Slab 0: k-outer with all 8 psum banks open, so each arriving
            # (w, x) k-slice pair immediately feeds 8 matmuls.
            ps0 = [psum_pool.tile([P, NT], FP32, tag="ps", name="ps") for _ in range(JT)]
            for t in range(KT):
                for j in range(JT):
                    nc.tensor.matmul(
                        ps0[j][:],
                        w_sb[:, t * D + j * P : t * D + (j + 1) * P],
                        x_sb0[:, t * NT : (t + 1) * NT],
                        start=(t == 0),
                        stop=(t == KT - 1),
                    )
            for j in range(JT):
                store(j, 0, ps0[j])

            # Remaining slabs: j-outer, psum banks recycle smoothly while the
            # next slab streams in.
            for bi in range(1, BT):
                x_sb = xpool.tile([P, KT * NT], x_dt, tag="x_sb0")
                for t in range(KT):
                    nc.sync.dma_start(
                        out=x_sb[:, t * NT : (t + 1) * NT],
                        in_=xT_v[:, t, bi * NT : (bi + 1) * NT],
                    )
                for j in range(JT):
                    ps = psum_pool.tile([P, NT], FP32, tag="ps")
                    for t in range(KT):
                        nc.tensor.matmul(
                            ps[:],
                            w_sb[:, t * D + j * P : t * D + (j + 1) * P],
                            x_sb[:, t * NT : (t + 1) * NT],
                            start=(t == 0),
                            stop=(t == KT - 1),
                        )
                    store(j, bi, ps)
    nc.finalize()
    return nc


def kernel(feat_rgb, feat_depth, W_k1, b_k1, W_k2, b_k2, W_f, b_f):
    global LAST_RESULT
    feat_rgb = np.asarray(feat_rgb, dtype=np.float32)
    feat_depth = np.asarray(feat_depth, dtype=np.float32)
    W_f = np.asarray(W_f, dtype=np.float32)
    b_f = np.asarray(b_f, dtype=np.float32)

    _, _, w_np, x_np = _dtypes()
    WfT = np.ascontiguousarray(W_f.T).astype(w_np)  # [2048, 1024]
    biasT = np.ascontiguousarray(b_f.reshape(JT, P).T)  # [128, 8]

    in_maps = []
    for i in range(NCORES):
        lo, hi = i * BLOC, (i + 1) * BLOC
        x_cat_T = np.empty((K, BLOC), dtype=x_np)
        x_cat_T[:D] = feat_depth[lo:hi].T
        x_cat_T[D:] = feat_rgb[lo:hi].T
        in_maps.append({"xT": x_cat_T, "wT": WfT, "biasT": biasT})

    nc = _build_nc()
    res = run_bass_kernel_spmd(
        nc, in_maps, list(range(NCORES)), trace=TRACE, tmpdir=TRACE_DIR
    )
    LAST_RESULT = res

    out = np.empty((B, D), dtype=np.float32)
    for i in range(NCORES):
        out[i * BLOC : (i + 1) * BLOC] = res.results[i]["yT"].T
    return out

